# revision 9
# baseline (speedup 1.0000x reference)
"""MoE (8 experts, top-2) Trainium2 kernel, expert-parallel across 8 NeuronCores.

v2 strategy (each core owns one expert, gate replicated):
  - Router logits in exact fp32 via bf16 hi/lo splitting, with [gw_hi|gw_lo]
    stacked into one 16-wide lhsT so terms (hi@xh + lo@xh) share one rhs
    stream; third term hi@xl streams x_lo. 2 rhs streams instead of 3.
  - Top-2 + renormalized gate weights on DVE (as before).
  - Compaction WITHOUT gpsimd sparse_gather: per-token slot ids via PE
    prefix-sum matmuls (strict-triangular ones), then the compacted token
    list is produced directly in ap_gather's wrapped [16, cap/16] layout by
    an accumulating matmul with data-dependent one-hot operands:
      idx[m, f] = sum_p onehot(slot%16 == m)[p] * (tokid * onehot(slot//16 == f))[p]
  - Dispatch via gpsimd ap_gather straight out of SBUF-resident fp32 x
    (assembled from the hi-stream during the router) -- no DRAM round trips,
    no indirect DMA, no PE transposes, and only ONE gpsimd library (loaded
    at kernel entry, off the critical path).
  - Expert MLP in bf16: (silu(x@w1) * (x@w3)) @ w2, scaled by gate coef.
  - Host combines: out[idx] += yT.T[:cnt].
"""
import sys

sys.path.insert(0, "/opt/trn_rl_repo")

import numpy as np

T, H, II, E = 2048, 1024, 4096, 8
P = 128
NT = T // P          # 16 token tiles
HC = H // P          # 8 hidden chunks
IC = II // P         # 32 intermediate chunks
NCORES = 8

_build_cache = {}


def _build(cap):
    """Build + schedule the per-core Tile kernel for token capacity `cap`."""
    import concourse.bass as bass
    import concourse.bacc as bacc
    import concourse.mybir as mybir
    from concourse.tile import TileContext

    f32 = mybir.dt.float32
    i32 = mybir.dt.int32
    i16 = mybir.dt.int16
    u32 = mybir.dt.uint32
    bf16 = mybir.dt.bfloat16
    fp16 = mybir.dt.float16
    AF = mybir.ActivationFunctionType
    OP = mybir.AluOpType

    _grp_table = {560: 2, 1024: 2, 2048: 4}
    assert cap in _grp_table, cap
    ngrp = _grp_table[cap]
    grp = cap // ngrp    # token group size per PSUM accumulation (<=512)
    cf = cap // 16       # wrapped free cols
    UNSEL = 32 * cf      # slot value for unselected tokens (sdiv=2*cf, no match)

    nc = bacc.Bacc("TRN2", target_bir_lowering=False)

    # ---- I/O ----
    xth = nc.declare_dram_parameter("xth", [H, T], bf16, isOutput=False)
    xtl = nc.declare_dram_parameter("xtl", [H, T], bf16, isOutput=False)
    GWS = 40  # [gw_hi (8) | zeros (24) | gw_lo (8)] -- lo lands at psum partitions 32:40
    gws = nc.declare_dram_parameter("gws", [H, GWS], bf16, isOutput=False)
    w1 = nc.declare_dram_parameter("w1", [H, II], bf16, isOutput=False)
    w3 = nc.declare_dram_parameter("w3", [H, II], bf16, isOutput=False)
    w2 = nc.declare_dram_parameter("w2", [II, H], bf16, isOutput=False)
    oh = nc.declare_dram_parameter("oh", [P, NT * E], f32, isOutput=False)
    tokid = nc.declare_dram_parameter("tokid", [P, NT], f32, isOutput=False)
    tokh_d = nc.declare_dram_parameter("tokh", [P, NT], fp16, isOutput=False)
    io16_d = nc.declare_dram_parameter("io16", [P, NT * 16], fp16, isOutput=False)
    iocf_d = nc.declare_dram_parameter("iocf", [P, NT * cf], fp16, isOutput=False)
    vt16_d = nc.declare_dram_parameter("vt16", [16, P], fp16, isOutput=False)
    stril_d = nc.declare_dram_parameter("stril", [P, P], bf16, isOutput=False)
    ones128_d = nc.declare_dram_parameter("ones128", [P, 1], bf16, isOutput=False)
    ident = nc.declare_dram_parameter("ident", [P, P], f32, isOutput=False)

    o_yt = nc.declare_dram_parameter("o_yt", [H, cap], f32, isOutput=True)
    o_idx = nc.declare_dram_parameter("o_idx", [cap], i32, isOutput=True)
    o_cnt = nc.declare_dram_parameter("o_cnt", [1, 1], u32, isOutput=True)

    with TileContext(nc) as tc:
        with (
            tc.tile_pool(name="sb", bufs=1) as sb,
            tc.tile_pool(name="sbw", bufs=2) as sbw,
            tc.tile_pool(name="psum", bufs=2, space="PSUM") as psg,
            tc.tile_pool(name="drp", bufs=1, space="DRAM") as drp,
        ):
            d_cf = drp.tile([cap], f32, tag="d_cf")

            # stacked gate weights [128, (hc, 40)]
            gw_s = sb.tile([P, HC * GWS], bf16, tag="gws")
            nc.sync.dma_start(
                out=gw_s[:].rearrange("p (hc e) -> p hc e", e=GWS),
                in_=gws[:].rearrange("(hc p) e -> p hc e", p=P),
            )

            warm_n = [0]

            def warm(dep_ap, kdim):
                n = min(dep_ap.shape[-1] if len(dep_ap.shape) == 2 else dep_ap.free_size(), 512)
                wps = psg.tile([8, n], f32, tag="mm3", name=f"warm{warm_n[0]}")
                warm_n[0] += 1
                nc.tensor.matmul(
                    out=wps[:],
                    lhsT=idt[0:kdim, 0:8],
                    rhs=dep_ap,
                    start=True, stop=True,
                )

            # ---- A. router: 2 rhs streams ----
            # ps_a[ng] [16,512] accumulates [gwh|gwl]^T @ x_hi
            # ps_b[ng] [8,512]  accumulates gwh^T @ x_lo
            ps_a = [psg.tile([40, 512], f32, tag=f"mm{ng}", name=f"psa{ng}") for ng in range(4)]
            ps_b = [psg.tile([E, 512], f32, tag=f"mm{ng}", name=f"psb{ng}") for ng in range(4)]
            xf32 = [sb.tile([P, T], f32, tag=f"xf{hc}", name=f"xf{hc}") for hc in range(HC)]
            for hc in range(HC):
                xt_h = sbw.tile([P, T], bf16, tag="xth", bufs=3)
                nc.sync.dma_start(out=xt_h[:], in_=xth[hc * P:(hc + 1) * P, :])
                xt_l = sbw.tile([P, T], bf16, tag="xtl", bufs=3)
                nc.sync.dma_start(out=xt_l[:], in_=xtl[hc * P:(hc + 1) * P, :])
                for ng in range(4):
                    nc.tensor.matmul(
                        out=ps_a[ng][:],
                        lhsT=gw_s[:, hc * GWS:(hc + 1) * GWS],
                        rhs=xt_h[:, ng * 512:(ng + 1) * 512],
                        start=(hc == 0), stop=(hc == HC - 1),
                    )
                    nc.tensor.matmul(
                        out=ps_b[ng][:],
                        lhsT=gw_s[:, hc * GWS:hc * GWS + E],
                        rhs=xt_l[:, ng * 512:(ng + 1) * 512],
                        start=(hc == 0), stop=(hc == HC - 1),
                    )
                # assemble fp32 x chunk (ap_gather source) from the hi part
                nc.vector.tensor_copy(out=xf32[hc][:], in_=xt_h[:])

            # ---- constants (loaded behind the x stream) ----
            idt = sb.tile([P, P], f32, tag="idt")
            nc.scalar.dma_start(out=idt[:], in_=ident[:])
            oh_sb = sb.tile([P, NT * E], f32, tag="oh")
            nc.scalar.dma_start(out=oh_sb[:], in_=oh[:])
            tk = sb.tile([P, NT], f32, tag="tk")
            nc.scalar.dma_start(out=tk[:], in_=tokid[:])
            tokh = sb.tile([P, NT], fp16, tag="tokh")
            nc.scalar.dma_start(out=tokh[:], in_=tokh_d[:])
            io16 = sb.tile([P, NT * 16], fp16, tag="io16")
            nc.scalar.dma_start(out=io16[:], in_=io16_d[:])
            iocf = sb.tile([P, NT * cf], fp16, tag="iocf")
            nc.scalar.dma_start(out=iocf[:], in_=iocf_d[:])
            vt16 = sb.tile([16, P], fp16, tag="vt16")
            nc.scalar.dma_start(out=vt16[:], in_=vt16_d[:])
            stril = sb.tile([P, P], bf16, tag="stril")
            nc.scalar.dma_start(out=stril[:], in_=stril_d[:])
            ones128 = sb.tile([P, 1], bf16, tag="ones128")
            nc.scalar.dma_start(out=ones128[:], in_=ones128_d[:])
            onesP = sb.tile([1, P], f32, tag="onesP")
            nc.vector.memset(onesP[:], 1.0)

            # fold: logitsT [8, T] = ps_a[0:8] + ps_a[8:16] + ps_b
            # (one PSUM operand per DVE op)
            logitsT = sb.tile([E, T], f32, tag="logitsT")
            for ng in range(4):
                gsl = slice(ng * 512, (ng + 1) * 512)
                nc.vector.tensor_copy(out=logitsT[:, gsl], in_=ps_a[ng][0:E, :])
                nc.vector.tensor_add(
                    out=logitsT[:, gsl],
                    in0=logitsT[:, gsl], in1=ps_a[ng][32:32 + E, :],
                )
                nc.vector.tensor_add(
                    out=logitsT[:, gsl],
                    in0=logitsT[:, gsl], in1=ps_b[ng][:],
                )

            # ---- B. transpose logitsT -> l_all [128, (16, 8)] ----
            l_all = sb.tile([P, NT * E], f32, tag="l_all")
            for ci in range(NT):
                tp = psg.tile([P, E], f32, tag="mm3")
                nc.tensor.transpose(
                    out=tp[:],
                    in_=logitsT[:, ci * P:(ci + 1) * P],
                    identity=idt[0:E, 0:E],
                )
                nc.vector.tensor_copy(out=l_all[:, ci * E:(ci + 1) * E], in_=tp[:])

            # ---- C. top-2 + coef ----
            l3 = l_all[:].rearrange("p (t e) -> p t e", e=E)
            m1a = sb.tile([P, NT, 4], f32, tag="m1a")
            m2a = sb.tile([P, NT, 4], f32, tag="m2a")
            nc.vector.tensor_tensor(out=m1a[:], in0=l3[:, :, 0::2], in1=l3[:, :, 1::2], op=OP.max)
            nc.vector.tensor_tensor(out=m2a[:], in0=l3[:, :, 0::2], in1=l3[:, :, 1::2], op=OP.min)
            m1b = sb.tile([P, NT, 2], f32, tag="m1b")
            m2b = sb.tile([P, NT, 2], f32, tag="m2b")
            tmin = sb.tile([P, NT, 2], f32, tag="tmin")
            nc.vector.tensor_tensor(out=m1b[:], in0=m1a[:, :, 0::2], in1=m1a[:, :, 1::2], op=OP.max)
            nc.vector.tensor_tensor(out=tmin[:], in0=m1a[:, :, 0::2], in1=m1a[:, :, 1::2], op=OP.min)
            nc.vector.tensor_tensor(out=m2b[:], in0=m2a[:, :, 0::2], in1=m2a[:, :, 1::2], op=OP.max)
            nc.vector.tensor_tensor(out=m2b[:], in0=m2b[:], in1=tmin[:], op=OP.max)
            m1 = sb.tile([P, NT, 1], f32, tag="m1")
            m2 = sb.tile([P, NT, 1], f32, tag="m2")
            tmin2 = sb.tile([P, NT, 1], f32, tag="tmin2")
            nc.vector.tensor_tensor(out=m1[:], in0=m1b[:, :, 0:1], in1=m1b[:, :, 1:2], op=OP.max)
            nc.vector.tensor_tensor(out=tmin2[:], in0=m1b[:, :, 0:1], in1=m1b[:, :, 1:2], op=OP.min)
            nc.vector.tensor_tensor(out=m2[:], in0=m2b[:, :, 0:1], in1=m2b[:, :, 1:2], op=OP.max)
            nc.vector.tensor_tensor(out=m2[:], in0=m2[:], in1=tmin2[:], op=OP.max)

            warm(m1[:, :, 0], P)
            dq = sb.tile([P, NT], f32, tag="dq")
            nc.vector.tensor_sub(out=dq[:], in0=m2[:, :, 0], in1=m1[:, :, 0])
            q = sb.tile([P, NT], f32, tag="q")
            nc.scalar.activation(out=q[:], in_=dq[:], func=AF.Exp)
            s = sb.tile([P, NT], f32, tag="s")
            nc.vector.tensor_scalar_add(s[:], q[:], 1.0)
            wt1 = sb.tile([P, NT], f32, tag="wt1")
            nc.vector.reciprocal(wt1[:], s[:])
            wt2 = sb.tile([P, NT], f32, tag="wt2")
            nc.vector.tensor_mul(out=wt2[:], in0=q[:], in1=wt1[:])

            le_m = sb.tile([P, NT, E], f32, tag="lem")
            nc.vector.tensor_mul(
                out=le_m[:], in0=l3[:], in1=oh_sb[:].rearrange("p (t e) -> p t e", e=E)
            )
            le = sb.tile([P, NT], f32, tag="le")
            nc.vector.reduce_sum(
                out=le[:].rearrange("p (t o) -> p t o", o=1),
                in_=le_m[:],
                axis=mybir.AxisListType.X,
            )

            eq1 = sb.tile([P, NT], f32, tag="eq1")
            eq2 = sb.tile([P, NT], f32, tag="eq2")
            nc.vector.tensor_tensor(out=eq1[:], in0=le[:], in1=m1[:, :, 0], op=OP.is_equal)
            nc.vector.tensor_tensor(out=eq2[:], in0=le[:], in1=m2[:, :, 0], op=OP.is_equal)
            coef = sb.tile([P, NT], f32, tag="coef")
            t1 = sb.tile([P, NT], f32, tag="t1")
            nc.vector.tensor_mul(out=coef[:], in0=eq1[:], in1=wt1[:])
            nc.vector.tensor_mul(out=t1[:], in0=eq2[:], in1=wt2[:])
            nc.vector.tensor_add(out=coef[:], in0=coef[:], in1=t1[:])
            selm = sb.tile([P, NT], f32, tag="selm")
            nc.vector.tensor_add(out=selm[:], in0=eq1[:], in1=eq2[:])
            warm(coef[:], P)

            # ---- D. slot assignment (column-major scan order) ----
            selm_bf = sb.tile([P, NT], bf16, tag="selmbf")
            nc.vector.tensor_copy(out=selm_bf[:], in_=selm[:])
            ps_pref = psg.tile([P, NT], f32, tag="mm0", name="pspref")
            nc.tensor.matmul(out=ps_pref[:], lhsT=stril[:], rhs=selm_bf[:], start=True, stop=True)
            ps_cs = psg.tile([1, NT], f32, tag="mm1", name="pscs")
            nc.tensor.matmul(out=ps_cs[:], lhsT=ones128[:], rhs=selm_bf[:], start=True, stop=True)
            # exclusive cumsum of column counts on [1,16]
            csA = sb.tile([1, NT], f32, tag="csA")
            nc.vector.tensor_copy(out=csA[:], in_=ps_cs[:])
            csB = sb.tile([1, NT], f32, tag="csB")
            cur = csA
            oth = csB
            for sh in (1, 2, 4, 8):
                nc.vector.tensor_copy(out=oth[:, 0:sh], in_=cur[:, 0:sh])
                nc.vector.tensor_add(out=oth[:, sh:NT], in0=cur[:, sh:NT], in1=cur[:, 0:NT - sh])
                cur, oth = oth, cur
            # cur = inclusive cumsum; cnt = cur[0, NT-1]
            cnt_u = sb.tile([1, 1], u32, tag="cntu")
            nc.vector.tensor_copy(out=cnt_u[:], in_=cur[:, NT - 1:NT])
            nc.scalar.dma_start(out=o_cnt[:], in_=cnt_u[:])
            o_excl = sb.tile([1, NT], f32, tag="oexcl")
            nc.vector.tensor_sub(out=o_excl[:], in0=cur[:], in1=ps_cs[:])
            ps_orep = psg.tile([P, NT], f32, tag="mm2", name="psorep")
            nc.tensor.matmul(out=ps_orep[:], lhsT=onesP[:], rhs=o_excl[:], start=True, stop=True)
            orep_sb = sb.tile([P, NT], f32, tag="orep")
            nc.vector.tensor_copy(out=orep_sb[:], in_=ps_orep[:])

            slot = sb.tile([P, NT], f32, tag="slot")
            nc.vector.tensor_add(out=slot[:], in0=ps_pref[:], in1=orep_sb[:])
            nc.vector.tensor_mul(out=slot[:], in0=slot[:], in1=selm[:])
            tsl = sb.tile([P, NT], f32, tag="tsl")
            nc.vector.tensor_scalar(tsl[:], selm[:], -float(UNSEL), float(UNSEL), op0=OP.mult, op1=OP.add)
            nc.vector.tensor_add(out=slot[:], in0=slot[:], in1=tsl[:])
            warm(slot[:], P)

            # sdiv = floor(slot/16) via round(slot/16 - 0.46875); smod = slot - 16*sdiv
            sd0 = sb.tile([P, NT], f32, tag="sd0")
            nc.vector.tensor_scalar(sd0[:], slot[:], 1.0 / 16.0, -0.46875, op0=OP.mult, op1=OP.add)
            sdi = sb.tile([P, NT], i32, tag="sdi")
            nc.vector.tensor_copy(out=sdi[:], in_=sd0[:])
            sdivf = sb.tile([P, NT], f32, tag="sdivf")
            nc.vector.tensor_copy(out=sdivf[:], in_=sdi[:])
            smf = sb.tile([P, NT], f32, tag="smf")
            nc.vector.tensor_scalar(smf[:], sdivf[:], -16.0, 0.0, op0=OP.mult, op1=OP.add)
            nc.vector.tensor_add(out=smf[:], in0=smf[:], in1=slot[:])
            sdivh = sb.tile([P, NT], fp16, tag="sdivh")
            nc.vector.tensor_copy(out=sdivh[:], in_=sdivf[:])
            smodh = sb.tile([P, NT], fp16, tag="smodh")
            nc.vector.tensor_copy(out=smodh[:], in_=smf[:])
            coefh = sb.tile([P, NT], fp16, tag="coefh")
            nc.vector.tensor_copy(out=coefh[:], in_=coef[:])

            # one-hot masks
            LH = sb.tile([P, NT * 16], fp16, tag="LH")
            nc.vector.tensor_tensor(
                out=LH[:].rearrange("p (c m) -> p c m", m=16),
                in0=io16[:].rearrange("p (c m) -> p c m", m=16),
                in1=smodh[:].rearrange("p (c o) -> p c o", o=1).to_broadcast([P, NT, 16]),
                op=OP.is_equal,
            )
            fmask = sb.tile([P, NT * cf], fp16, tag="fmask")
            nc.vector.tensor_tensor(
                out=fmask[:].rearrange("p (c f) -> p c f", f=cf),
                in0=iocf[:].rearrange("p (c f) -> p c f", f=cf),
                in1=sdivh[:].rearrange("p (c o) -> p c o", o=1).to_broadcast([P, NT, cf]),
                op=OP.is_equal,
            )
            RHS = sb.tile([P, NT * 2 * cf], fp16, tag="RHS")
            rhs4 = RHS[:].rearrange("p (c z f) -> p c z f", z=2, f=cf)
            fm4 = fmask[:].rearrange("p (c o f) -> p c o f", o=1, f=cf)
            nc.vector.tensor_tensor(
                out=rhs4[:, :, 0:1, :],
                in0=fm4[:],
                in1=tokh[:].rearrange("p (c o w) -> p c o w", o=1, w=1)
                    .to_broadcast([P, NT, 1, cf]),
                op=OP.mult,
            )
            nc.vector.tensor_tensor(
                out=rhs4[:, :, 1:2, :],
                in0=fm4[:],
                in1=coefh[:].rearrange("p (c o w) -> p c o w", o=1, w=1)
                    .to_broadcast([P, NT, 1, cf]),
                op=OP.mult,
            )

            # inversion matmuls: inv [16, 2*cf] = sum_c LH_c^T @ RHS_c
            inv = psg.tile([16, 2 * cf], f32, tag="mm3", name="inv")
            for c in range(NT):
                nc.tensor.matmul(
                    out=inv[:],
                    lhsT=LH[:, c * 16:(c + 1) * 16],
                    rhs=RHS[:, c * 2 * cf:(c + 1) * 2 * cf],
                    start=(c == 0), stop=(c == NT - 1),
                )
            coefw = sb.tile([16, cf], f32, tag="coefw")
            nc.vector.tensor_copy(out=coefw[:], in_=inv[:, cf:2 * cf])
            fill = sb.tile([16, cf], f32, tag="fill")
            nc.vector.tensor_scalar(fill[:], coefw[:], 0.0, None, op0=OP.is_gt)
            idx_f = sb.tile([16, cf], f32, tag="idxf")
            nc.vector.tensor_add(out=idx_f[:], in0=inv[:, 0:cf], in1=fill[:])
            nc.vector.tensor_scalar_sub(idx_f[:], idx_f[:], 1.0)

            idx_i = sb.tile([16, cf], i32, tag="idxi")
            nc.vector.tensor_copy(out=idx_i[:], in_=idx_f[:])
            nc.scalar.dma_start(out=o_idx[:].rearrange("(f p) -> p f", p=16), in_=idx_i[:])

            # replicate wrapped idx to all 8 core groups -> int16
            idxh = sb.tile([16, cf], fp16, tag="idxh")
            nc.vector.tensor_copy(out=idxh[:], in_=idx_f[:])
            ps_rep = psg.tile([P, cf], f32, tag="mm0", name="psrep")
            nc.tensor.matmul(out=ps_rep[:], lhsT=vt16[:], rhs=idxh[:], start=True, stop=True)
            idxs16 = sb.tile([P, cf], i16, tag="idxs16")
            nc.vector.tensor_copy(out=idxs16[:], in_=ps_rep[:])
            warm(slot[:], P)

            # ---- E. dispatch: gather selected token columns from resident x ----
            xgT = [sb.tile([P, cap], bf16, tag=f"xgT{hc}", name=f"xgT{hc}") for hc in range(HC)]
            for hc in range(HC):
                xg32 = sbw.tile([P, cap], f32, tag="xg32", bufs=3)
                nc.gpsimd.ap_gather(
                    out_ap=xg32[:], in_ap=xf32[hc][:], idxs_ap=idxs16[:],
                    channels=P, num_elems=T, d=1, num_idxs=cap,
                )
                nc.vector.tensor_copy(out=xgT[hc][:], in_=xg32[:])

            # ---- F. coef broadcast [128, cap] via dense DRAM round trip ----
            tpc = psg.tile([cf, 16], f32, tag="mm1", name="tpc")
            nc.tensor.transpose(out=tpc[:], in_=coefw[:], identity=idt[0:16, 0:16])
            ct = sb.tile([cf, 16], f32, tag="ct")
            nc.vector.tensor_copy(out=ct[:], in_=tpc[:])
            nc.scalar.dma_start(out=d_cf[:].rearrange("(f p) -> f p", p=16), in_=ct[:])
            vrow = sb.tile([1, cap], f32, tag="vrow")
            nc.scalar.dma_start(out=vrow[:], in_=d_cf[:].rearrange("(o c) -> o c", o=1))
            cbc = sb.tile([P, cap], f32, tag="cbc")
            for g in range(ngrp):
                cb_ps = psg.tile([P, grp], f32, tag="mm2")
                nc.tensor.matmul(
                    out=cb_ps[:], lhsT=onesP[:],
                    rhs=vrow[:, g * grp:(g + 1) * grp], start=True, stop=True,
                )
                nc.vector.tensor_copy(out=cbc[:, g * grp:(g + 1) * grp], in_=cb_ps[:])

            # ---- G. h1 = x@w1, h3 = x@w3 (transposed), fused silu*mul ----
            actT = [sb.tile([P, cap], bf16, tag=f"actT{ic}", name=f"actT{ic}") for ic in range(IC)]
            for ic in range(IC):
                w1_sl = sbw.tile([P, H], bf16, tag="w1sl", bufs=4)
                nc.sync.dma_start(
                    out=w1_sl[:].rearrange("p (hc i) -> p hc i", i=P),
                    in_=w1[:, ic * P:(ic + 1) * P].rearrange("(hc p) i -> p hc i", p=P),
                )
                w3_sl = sbw.tile([P, H], bf16, tag="w3sl", bufs=4)
                nc.sync.dma_start(
                    out=w3_sl[:].rearrange("p (hc i) -> p hc i", i=P),
                    in_=w3[:, ic * P:(ic + 1) * P].rearrange("(hc p) i -> p hc i", p=P),
                )
                for g in range(ngrp):
                    gs = slice(g * grp, (g + 1) * grp)
                    ps1 = psg.tile([P, grp], f32, tag="mm0")
                    ps3 = psg.tile([P, grp], f32, tag="mm1")
                    for hc in range(HC):
                        nc.tensor.matmul(
                            out=ps1[:],
                            lhsT=w1_sl[:, hc * P:(hc + 1) * P],
                            rhs=xgT[hc][:, gs],
                            start=(hc == 0), stop=(hc == HC - 1),
                        )
                    for hc in range(HC):
                        nc.tensor.matmul(
                            out=ps3[:],
                            lhsT=w3_sl[:, hc * P:(hc + 1) * P],
                            rhs=xgT[hc][:, gs],
                            start=(hc == 0), stop=(hc == HC - 1),
                        )
                    sl = sbw.tile([P, grp], f32, tag="silu")
                    nc.scalar.activation(out=sl[:], in_=ps1[:], func=AF.Silu)
                    nc.vector.tensor_mul(out=actT[ic][:, gs], in0=sl[:], in1=ps3[:])

            # ---- H. yT = (act @ w2).T * coef ----
            for hc in range(HC):
                w2_sl = sbw.tile([P, II], bf16, tag="w2sl", bufs=3)
                nc.sync.dma_start(
                    out=w2_sl[:].rearrange("p (ic h) -> p ic h", h=P),
                    in_=w2[:, hc * P:(hc + 1) * P].rearrange("(ic p) h -> p ic h", p=P),
                )
                for g in range(ngrp):
                    gs = slice(g * grp, (g + 1) * grp)
                    pso = psg.tile([P, grp], f32, tag="mm2")
                    for ic in range(IC):
                        nc.tensor.matmul(
                            out=pso[:],
                            lhsT=w2_sl[:, ic * P:(ic + 1) * P],
                            rhs=actT[ic][:, gs],
                            start=(ic == 0), stop=(ic == IC - 1),
                        )
                    yt_sb = sbw.tile([P, grp], f32, tag="yt")
                    nc.vector.tensor_mul(out=yt_sb[:], in0=pso[:], in1=cbc[:, gs])
                    nc.sync.dma_start(
                        out=o_yt[hc * P:(hc + 1) * P, gs], in_=yt_sb[:]
                    )

    nc.compile()
    return nc


def _get_built(cap):
    if cap not in _build_cache:
        _build_cache[cap] = _build(cap)
    return _build_cache[cap]


def _make_consts(cap):
    cf = cap // 16
    tokid_np = (np.arange(NT)[None, :] * P + np.arange(P)[:, None]).astype(np.float32)
    io16_np = np.broadcast_to(
        np.tile(np.arange(16), NT)[None, :], (P, NT * 16)
    ).astype(np.float16)
    iocf_np = np.broadcast_to(
        np.tile(np.arange(cf), NT)[None, :], (P, NT * cf)
    ).astype(np.float16)
    vt16_np = np.zeros((16, P), np.float16)
    for u in range(8):
        for p in range(16):
            vt16_np[p, 16 * u + p] = 1.0
    stril_np = np.triu(np.ones((P, P), np.float32), 1)  # [k, m] = 1 if k < m
    ones128_np = np.ones((P, 1), np.float32)
    ident_np = np.eye(P, dtype=np.float32)
    return tokid_np, io16_np, iocf_np, vt16_np, stril_np, ones128_np, ident_np


def _run(cap, hs, gate_w, w1s, w2s, w3s, trace=False):
    import ml_dtypes
    from concourse.bass_utils import run_bass_kernel_spmd

    nc = _get_built(cap)

    bf = ml_dtypes.bfloat16
    x_hi = hs.astype(bf)
    x_lo = (hs - x_hi.astype(np.float32)).astype(bf)
    xth_np = np.ascontiguousarray(x_hi.T)
    xtl_np = np.ascontiguousarray(x_lo.T)
    gw_hi = gate_w.astype(bf)
    gw_lo = (gate_w - gw_hi.astype(np.float32)).astype(bf)
    gws_np = np.zeros((H, 40), np.float32)
    gws_np[:, 0:E] = gw_hi.astype(np.float32)
    gws_np[:, 32:40] = gw_lo.astype(np.float32)
    gws_np = np.ascontiguousarray(gws_np.astype(bf))
    oh_base = np.zeros((P, NT, E), np.float32)
    tokid_np, io16_np, iocf_np, vt16_np, stril_np, ones128_np, ident_np = _make_consts(cap)

    in_maps = []
    for c in range(NCORES):
        oh_c = oh_base.copy()
        oh_c[:, :, c] = 1.0
        in_maps.append({
            "xth": xth_np,
            "xtl": xtl_np,
            "gws": gws_np,
            "w1": np.ascontiguousarray(w1s[c].astype(bf)),
            "w3": np.ascontiguousarray(w3s[c].astype(bf)),
            "w2": np.ascontiguousarray(w2s[c].astype(bf)),
            "oh": oh_c.reshape(P, NT * E),
            "tokid": tokid_np,
            "tokh": tokid_np.astype(np.float16),
            "io16": io16_np,
            "iocf": iocf_np,
            "vt16": vt16_np,
            "stril": stril_np.astype(bf),
            "ones128": ones128_np.astype(bf),
            "ident": ident_np,
        })

    res = run_bass_kernel_spmd(nc, in_maps, list(range(NCORES)), trace=trace)
    return res


def kernel(hidden_states, gate_w, w1s, w2s, w3s, _trace=False, _cap=560):
    hs = np.ascontiguousarray(np.asarray(hidden_states, dtype=np.float32))
    gate_w = np.ascontiguousarray(np.asarray(gate_w, dtype=np.float32))
    w1s = np.asarray(w1s, dtype=np.float32)
    w2s = np.asarray(w2s, dtype=np.float32)
    w3s = np.asarray(w3s, dtype=np.float32)

    cap = _cap
    while True:
        res = _run(cap, hs, gate_w, w1s, w2s, w3s, trace=_trace)
        counts = [int(res.results[c]["o_cnt"].ravel()[0]) for c in range(NCORES)]
        if max(counts) <= cap:
            break
        # capacity overflow (won't happen for sane routing): rebuild bigger
        cap = 2048 if max(counts) > 1024 else 1024

    out = np.zeros((T, H), dtype=np.float32)
    for c in range(NCORES):
        r = res.results[c]
        cnt = counts[c]
        idx = r["o_idx"][:cnt]
        y = np.ascontiguousarray(r["o_yt"].T[:cnt])
        out[idx] += y
    kernel._last_results = res
    return out


# revision 13
# speedup vs baseline: 1.3710x; 1.3710x over previous
"""MoE (8 experts, top-2) Trainium2 kernel, expert-parallel across 8 NeuronCores.

v2 strategy (each core owns one expert, gate replicated):
  - Router logits in exact fp32 via bf16 hi/lo splitting, with [gw_hi|gw_lo]
    stacked into one 16-wide lhsT so terms (hi@xh + lo@xh) share one rhs
    stream; third term hi@xl streams x_lo. 2 rhs streams instead of 3.
  - Top-2 + renormalized gate weights on DVE (as before).
  - Compaction WITHOUT gpsimd sparse_gather: per-token slot ids via PE
    prefix-sum matmuls (strict-triangular ones), then the compacted token
    list is produced directly in partition-major [128, ceil(cap/128)] layout
    by an accumulating matmul with data-dependent one-hot operands:
      idx[r, k] = sum_p onehot(slot%128 == r)[p] * (tokid * onehot(slot//128 == k))[p]
    This also yields the per-slot coef in the same pass. No gpsimd custom
    libraries, no DRAM round trips for the index relayout.
  - Dispatch via indirect-DMA row gather of x (bf16, row major) using the
    [128,1] offset columns straight from the inversion output, then PE
    transposes into [H-chunk, slot] layout.
  - Expert MLP in bf16: (silu(x@w1) * (x@w3)) @ w2, scaled by gate coef.
  - Host combines: out[idx] += yT.T[:cnt].
"""
import sys

sys.path.insert(0, "/opt/trn_rl_repo")

import numpy as np

T, H, II, E = 2048, 1024, 4096, 8
P = 128
NT = T // P          # 16 token tiles
HC = H // P          # 8 hidden chunks
IC = II // P         # 32 intermediate chunks
NCORES = 8

_build_cache = {}


def _build(cap):
    """Build + schedule the per-core Tile kernel for token capacity `cap`."""
    import concourse.bass as bass
    import concourse.bacc as bacc
    import concourse.mybir as mybir
    from concourse.tile import TileContext

    f32 = mybir.dt.float32
    i32 = mybir.dt.int32
    i16 = mybir.dt.int16
    u32 = mybir.dt.uint32
    bf16 = mybir.dt.bfloat16
    fp16 = mybir.dt.float16
    AF = mybir.ActivationFunctionType
    OP = mybir.AluOpType

    _grp_table = {560: 2, 1024: 2, 2048: 4}
    assert cap in _grp_table, cap
    ngrp = _grp_table[cap]
    grp = cap // ngrp    # token group size per PSUM accumulation (<=512)
    cf = cap // 16       # wrapped free cols (o_idx layout only)
    NFT = (cap + P - 1) // P   # gather tiles (partition-major slot columns)
    rem = cap - (NFT - 1) * P  # rows in last gather tile
    UNSEL = 4 * NFT * P  # slot for unselected tokens (sdiv out of range)

    nc = bacc.Bacc("TRN2", target_bir_lowering=False)

    # ---- I/O ----
    xth = nc.declare_dram_parameter("xth", [H, T], bf16, isOutput=False)
    xtl = nc.declare_dram_parameter("xtl", [H, T], bf16, isOutput=False)
    GWS = 40  # [gw_hi (8) | zeros (24) | gw_lo (8)] -- lo lands at psum partitions 32:40
    gws = nc.declare_dram_parameter("gws", [H, GWS], bf16, isOutput=False)
    w1 = nc.declare_dram_parameter("w1", [H, II], bf16, isOutput=False)
    w3 = nc.declare_dram_parameter("w3", [H, II], bf16, isOutput=False)
    w2 = nc.declare_dram_parameter("w2", [II, H], bf16, isOutput=False)
    oh = nc.declare_dram_parameter("oh", [P, NT * E], f32, isOutput=False)
    tokid = nc.declare_dram_parameter("tokid", [P, NT], f32, isOutput=False)
    tokh_d = nc.declare_dram_parameter("tokh", [P, NT], fp16, isOutput=False)
    x_d = nc.declare_dram_parameter("x", [T, H], bf16, isOutput=False)
    io128_d = nc.declare_dram_parameter("io128", [P, NT * P], fp16, isOutput=False)
    io5_d = nc.declare_dram_parameter("io5", [P, NT * NFT], fp16, isOutput=False)
    stril_d = nc.declare_dram_parameter("stril", [P, P], bf16, isOutput=False)
    ones128_d = nc.declare_dram_parameter("ones128", [P, 1], bf16, isOutput=False)
    ident = nc.declare_dram_parameter("ident", [P, P], f32, isOutput=False)

    o_yt = nc.declare_dram_parameter("o_yt", [H, cap], f32, isOutput=True)
    o_idx = nc.declare_dram_parameter("o_idx", [cap], i32, isOutput=True)
    o_cnt = nc.declare_dram_parameter("o_cnt", [1, 1], u32, isOutput=True)

    with TileContext(nc) as tc:
        with (
            tc.tile_pool(name="sb", bufs=1) as sb,
            tc.tile_pool(name="sbw", bufs=2) as sbw,
            tc.tile_pool(name="psum", bufs=2, space="PSUM") as psg,
            tc.tile_pool(name="drp", bufs=1, space="DRAM") as drp,
        ):
            d_cf = drp.tile([NFT * P], f32, tag="d_cf")

            # stacked gate weights [128, (hc, 40)]
            gw_s = sb.tile([P, HC * GWS], bf16, tag="gws")
            nc.sync.dma_start(
                out=gw_s[:].rearrange("p (hc e) -> p hc e", e=GWS),
                in_=gws[:].rearrange("(hc p) e -> p hc e", p=P),
            )

            warm_n = [0]

            def warm(dep_ap, kdim):
                n = min(dep_ap.shape[-1] if len(dep_ap.shape) == 2 else dep_ap.free_size(), 512)
                wps = psg.tile([8, n], f32, tag="mm3", name=f"warm{warm_n[0]}")
                warm_n[0] += 1
                nc.tensor.matmul(
                    out=wps[:],
                    lhsT=idt[0:kdim, 0:8],
                    rhs=dep_ap,
                    start=True, stop=True,
                )

            # ---- A. router: 2 rhs streams ----
            # ps_a[ng] [16,512] accumulates [gwh|gwl]^T @ x_hi
            # ps_b[ng] [8,512]  accumulates gwh^T @ x_lo
            ps_a = [psg.tile([40, 512], f32, tag=f"mm{ng}", name=f"psa{ng}") for ng in range(4)]
            ps_b = [psg.tile([E, 512], f32, tag=f"mm{ng}", name=f"psb{ng}") for ng in range(4)]
            for hc in range(HC):
                xt_h = sbw.tile([P, T], bf16, tag="xth", bufs=3)
                nc.sync.dma_start(out=xt_h[:], in_=xth[hc * P:(hc + 1) * P, :])
                xt_l = sbw.tile([P, T], bf16, tag="xtl", bufs=3)
                nc.sync.dma_start(out=xt_l[:], in_=xtl[hc * P:(hc + 1) * P, :])
                for ng in range(4):
                    nc.tensor.matmul(
                        out=ps_a[ng][:],
                        lhsT=gw_s[:, hc * GWS:(hc + 1) * GWS],
                        rhs=xt_h[:, ng * 512:(ng + 1) * 512],
                        start=(hc == 0), stop=(hc == HC - 1),
                    )
                    nc.tensor.matmul(
                        out=ps_b[ng][:],
                        lhsT=gw_s[:, hc * GWS:hc * GWS + E],
                        rhs=xt_l[:, ng * 512:(ng + 1) * 512],
                        start=(hc == 0), stop=(hc == HC - 1),
                    )


            # ---- constants (loaded behind the x stream) ----
            idt = sb.tile([P, P], f32, tag="idt")
            nc.scalar.dma_start(out=idt[:], in_=ident[:])
            oh_sb = sb.tile([P, NT * E], f32, tag="oh")
            nc.scalar.dma_start(out=oh_sb[:], in_=oh[:])
            tk = sb.tile([P, NT], f32, tag="tk")
            nc.scalar.dma_start(out=tk[:], in_=tokid[:])
            tokh = sb.tile([P, NT], fp16, tag="tokh")
            nc.scalar.dma_start(out=tokh[:], in_=tokh_d[:])
            io128 = sb.tile([P, NT * P], fp16, tag="io128")
            nc.scalar.dma_start(out=io128[:], in_=io128_d[:])
            io5 = sb.tile([P, NT * NFT], fp16, tag="io5")
            nc.scalar.dma_start(out=io5[:], in_=io5_d[:])
            stril = sb.tile([P, P], bf16, tag="stril")
            nc.scalar.dma_start(out=stril[:], in_=stril_d[:])
            ones128 = sb.tile([P, 1], bf16, tag="ones128")
            nc.scalar.dma_start(out=ones128[:], in_=ones128_d[:])
            onesP = sb.tile([1, P], f32, tag="onesP")
            nc.vector.memset(onesP[:], 1.0)
            idtb = sb.tile([P, P], bf16, tag="idtb")
            nc.vector.tensor_copy(out=idtb[:], in_=idt[:])

            # fold: logitsT [8, T] = ps_a[0:8] + ps_a[8:16] + ps_b
            # (one PSUM operand per DVE op)
            logitsT = sb.tile([E, T], f32, tag="logitsT")
            for ng in range(4):
                gsl = slice(ng * 512, (ng + 1) * 512)
                nc.vector.tensor_copy(out=logitsT[:, gsl], in_=ps_a[ng][0:E, :])
                nc.vector.tensor_add(
                    out=logitsT[:, gsl],
                    in0=logitsT[:, gsl], in1=ps_a[ng][32:32 + E, :],
                )
                nc.vector.tensor_add(
                    out=logitsT[:, gsl],
                    in0=logitsT[:, gsl], in1=ps_b[ng][:],
                )

            # ---- B. transpose logitsT -> l_all [128, (16, 8)] ----
            l_all = sb.tile([P, NT * E], f32, tag="l_all")
            for ci in range(NT):
                tp = psg.tile([P, E], f32, tag="mm3")
                nc.tensor.transpose(
                    out=tp[:],
                    in_=logitsT[:, ci * P:(ci + 1) * P],
                    identity=idt[0:E, 0:E],
                )
                nc.vector.tensor_copy(out=l_all[:, ci * E:(ci + 1) * E], in_=tp[:])

            # ---- C. top-2 + coef ----
            l3 = l_all[:].rearrange("p (t e) -> p t e", e=E)
            m1a = sb.tile([P, NT, 4], f32, tag="m1a")
            m2a = sb.tile([P, NT, 4], f32, tag="m2a")
            nc.vector.tensor_tensor(out=m1a[:], in0=l3[:, :, 0::2], in1=l3[:, :, 1::2], op=OP.max)
            nc.vector.tensor_tensor(out=m2a[:], in0=l3[:, :, 0::2], in1=l3[:, :, 1::2], op=OP.min)
            m1b = sb.tile([P, NT, 2], f32, tag="m1b")
            m2b = sb.tile([P, NT, 2], f32, tag="m2b")
            tmin = sb.tile([P, NT, 2], f32, tag="tmin")
            nc.vector.tensor_tensor(out=m1b[:], in0=m1a[:, :, 0::2], in1=m1a[:, :, 1::2], op=OP.max)
            nc.vector.tensor_tensor(out=tmin[:], in0=m1a[:, :, 0::2], in1=m1a[:, :, 1::2], op=OP.min)
            nc.vector.tensor_tensor(out=m2b[:], in0=m2a[:, :, 0::2], in1=m2a[:, :, 1::2], op=OP.max)
            nc.vector.tensor_tensor(out=m2b[:], in0=m2b[:], in1=tmin[:], op=OP.max)
            m1 = sb.tile([P, NT, 1], f32, tag="m1")
            m2 = sb.tile([P, NT, 1], f32, tag="m2")
            tmin2 = sb.tile([P, NT, 1], f32, tag="tmin2")
            nc.vector.tensor_tensor(out=m1[:], in0=m1b[:, :, 0:1], in1=m1b[:, :, 1:2], op=OP.max)
            nc.vector.tensor_tensor(out=tmin2[:], in0=m1b[:, :, 0:1], in1=m1b[:, :, 1:2], op=OP.min)
            nc.vector.tensor_tensor(out=m2[:], in0=m2b[:, :, 0:1], in1=m2b[:, :, 1:2], op=OP.max)
            nc.vector.tensor_tensor(out=m2[:], in0=m2[:], in1=tmin2[:], op=OP.max)

            warm(m1[:, :, 0], P)
            dq = sb.tile([P, NT], f32, tag="dq")
            nc.vector.tensor_sub(out=dq[:], in0=m2[:, :, 0], in1=m1[:, :, 0])
            q = sb.tile([P, NT], f32, tag="q")
            nc.scalar.activation(out=q[:], in_=dq[:], func=AF.Exp)
            s = sb.tile([P, NT], f32, tag="s")
            nc.vector.tensor_scalar_add(s[:], q[:], 1.0)
            wt1 = sb.tile([P, NT], f32, tag="wt1")
            nc.vector.reciprocal(wt1[:], s[:])
            wt2 = sb.tile([P, NT], f32, tag="wt2")
            nc.vector.tensor_mul(out=wt2[:], in0=q[:], in1=wt1[:])

            le_m = sb.tile([P, NT, E], f32, tag="lem")
            nc.vector.tensor_mul(
                out=le_m[:], in0=l3[:], in1=oh_sb[:].rearrange("p (t e) -> p t e", e=E)
            )
            le = sb.tile([P, NT], f32, tag="le")
            nc.vector.reduce_sum(
                out=le[:].rearrange("p (t o) -> p t o", o=1),
                in_=le_m[:],
                axis=mybir.AxisListType.X,
            )

            eq1 = sb.tile([P, NT], f32, tag="eq1")
            eq2 = sb.tile([P, NT], f32, tag="eq2")
            nc.vector.tensor_tensor(out=eq1[:], in0=le[:], in1=m1[:, :, 0], op=OP.is_equal)
            nc.vector.tensor_tensor(out=eq2[:], in0=le[:], in1=m2[:, :, 0], op=OP.is_equal)
            coef = sb.tile([P, NT], f32, tag="coef")
            t1 = sb.tile([P, NT], f32, tag="t1")
            nc.vector.tensor_mul(out=coef[:], in0=eq1[:], in1=wt1[:])
            nc.vector.tensor_mul(out=t1[:], in0=eq2[:], in1=wt2[:])
            nc.vector.tensor_add(out=coef[:], in0=coef[:], in1=t1[:])
            selm = sb.tile([P, NT], f32, tag="selm")
            nc.vector.tensor_add(out=selm[:], in0=eq1[:], in1=eq2[:])
            warm(coef[:], P)

            # ---- D. slot assignment (column-major scan order) ----
            selm_bf = sb.tile([P, NT], bf16, tag="selmbf")
            nc.vector.tensor_copy(out=selm_bf[:], in_=selm[:])
            ps_pref = psg.tile([P, NT], f32, tag="mm0", name="pspref")
            nc.tensor.matmul(out=ps_pref[:], lhsT=stril[:], rhs=selm_bf[:], start=True, stop=True)
            ps_cs = psg.tile([1, NT], f32, tag="mm1", name="pscs")
            nc.tensor.matmul(out=ps_cs[:], lhsT=ones128[:], rhs=selm_bf[:], start=True, stop=True)
            # exclusive cumsum of column counts on [1,16]
            csA = sb.tile([1, NT], f32, tag="csA")
            nc.vector.tensor_copy(out=csA[:], in_=ps_cs[:])
            csB = sb.tile([1, NT], f32, tag="csB")
            cur = csA
            oth = csB
            for sh in (1, 2, 4, 8):
                nc.vector.tensor_copy(out=oth[:, 0:sh], in_=cur[:, 0:sh])
                nc.vector.tensor_add(out=oth[:, sh:NT], in0=cur[:, sh:NT], in1=cur[:, 0:NT - sh])
                cur, oth = oth, cur
            # cur = inclusive cumsum; cnt = cur[0, NT-1]
            cnt_u = sb.tile([1, 1], u32, tag="cntu")
            nc.vector.tensor_copy(out=cnt_u[:], in_=cur[:, NT - 1:NT])
            nc.scalar.dma_start(out=o_cnt[:], in_=cnt_u[:])
            o_excl = sb.tile([1, NT], f32, tag="oexcl")
            nc.vector.tensor_sub(out=o_excl[:], in0=cur[:], in1=ps_cs[:])
            ps_orep = psg.tile([P, NT], f32, tag="mm2", name="psorep")
            nc.tensor.matmul(out=ps_orep[:], lhsT=onesP[:], rhs=o_excl[:], start=True, stop=True)
            orep_sb = sb.tile([P, NT], f32, tag="orep")
            nc.vector.tensor_copy(out=orep_sb[:], in_=ps_orep[:])

            slot = sb.tile([P, NT], f32, tag="slot")
            nc.vector.tensor_add(out=slot[:], in0=ps_pref[:], in1=orep_sb[:])
            nc.vector.tensor_mul(out=slot[:], in0=slot[:], in1=selm[:])
            tsl = sb.tile([P, NT], f32, tag="tsl")
            nc.vector.tensor_scalar(tsl[:], selm[:], -float(UNSEL), float(UNSEL), op0=OP.mult, op1=OP.add)
            nc.vector.tensor_add(out=slot[:], in0=slot[:], in1=tsl[:])
            warm(slot[:], P)

            # sdiv = floor(slot/128) via round(slot/128 - bias); smod = slot - 128*sdiv
            sd0 = sb.tile([P, NT], f32, tag="sd0")
            nc.vector.tensor_scalar(sd0[:], slot[:], 1.0 / 128.0, -0.49609375, op0=OP.mult, op1=OP.add)
            sdi = sb.tile([P, NT], i32, tag="sdi")
            nc.vector.tensor_copy(out=sdi[:], in_=sd0[:])
            sdivf = sb.tile([P, NT], f32, tag="sdivf")
            nc.vector.tensor_copy(out=sdivf[:], in_=sdi[:])
            smf = sb.tile([P, NT], f32, tag="smf")
            nc.vector.tensor_scalar(smf[:], sdivf[:], -128.0, 0.0, op0=OP.mult, op1=OP.add)
            nc.vector.tensor_add(out=smf[:], in0=smf[:], in1=slot[:])
            sdivh = sb.tile([P, NT], fp16, tag="sdivh")
            nc.vector.tensor_copy(out=sdivh[:], in_=sdivf[:])
            smodh = sb.tile([P, NT], fp16, tag="smodh")
            nc.vector.tensor_copy(out=smodh[:], in_=smf[:])
            coefh = sb.tile([P, NT], fp16, tag="coefh")
            nc.vector.tensor_copy(out=coefh[:], in_=coef[:])

            # one-hot masks
            LH = sb.tile([P, NT * P], fp16, tag="LH")
            nc.vector.tensor_tensor(
                out=LH[:].rearrange("p (c m) -> p c m", m=P),
                in0=io128[:].rearrange("p (c m) -> p c m", m=P),
                in1=smodh[:].rearrange("p (c o) -> p c o", o=1).to_broadcast([P, NT, P]),
                op=OP.is_equal,
            )
            fmask = sb.tile([P, NT * NFT], fp16, tag="fmask")
            nc.vector.tensor_tensor(
                out=fmask[:].rearrange("p (c f) -> p c f", f=NFT),
                in0=io5[:].rearrange("p (c f) -> p c f", f=NFT),
                in1=sdivh[:].rearrange("p (c o) -> p c o", o=1).to_broadcast([P, NT, NFT]),
                op=OP.is_equal,
            )
            RHS = sb.tile([P, NT * 2 * NFT], fp16, tag="RHS")
            rhs4 = RHS[:].rearrange("p (c z f) -> p c z f", z=2, f=NFT)
            fm4 = fmask[:].rearrange("p (c o f) -> p c o f", o=1, f=NFT)
            nc.vector.tensor_tensor(
                out=rhs4[:, :, 0:1, :],
                in0=fm4[:],
                in1=tokh[:].rearrange("p (c o w) -> p c o w", o=1, w=1)
                    .to_broadcast([P, NT, 1, NFT]),
                op=OP.mult,
            )
            nc.vector.tensor_tensor(
                out=rhs4[:, :, 1:2, :],
                in0=fm4[:],
                in1=coefh[:].rearrange("p (c o w) -> p c o w", o=1, w=1)
                    .to_broadcast([P, NT, 1, NFT]),
                op=OP.mult,
            )

            # inversion matmuls: inv [128, 2*NFT] = sum_c LH_c^T @ RHS_c
            # inv[r, k]      = token id at slot k*128+r  (0 if unfilled)
            # inv[r, NFT+k]  = gate coef of slot k*128+r (0 if unfilled)
            inv = psg.tile([P, 2 * NFT], f32, tag="mm3", name="inv")
            for c in range(NT):
                nc.tensor.matmul(
                    out=inv[:],
                    lhsT=LH[:, c * P:(c + 1) * P],
                    rhs=RHS[:, c * 2 * NFT:(c + 1) * 2 * NFT],
                    start=(c == 0), stop=(c == NT - 1),
                )
            idxp = sb.tile([P, NFT], i32, tag="idxp")
            nc.vector.tensor_copy(out=idxp[:], in_=inv[:, 0:NFT])
            coefp = sb.tile([P, NFT], f32, tag="coefp")
            nc.vector.tensor_copy(out=coefp[:], in_=inv[:, NFT:2 * NFT])
            warm(slot[:], P)

            # o_idx (host reads first cnt entries)
            nc.scalar.dma_start(
                out=o_idx[0:(NFT - 1) * P].rearrange("(k p) -> p k", p=P),
                in_=idxp[:, 0:NFT - 1],
            )
            nc.scalar.dma_start(
                out=o_idx[(NFT - 1) * P:cap].rearrange("(p o) -> p o", o=1),
                in_=idxp[0:rem, NFT - 1:NFT],
            )

            # ---- E. dispatch: indirect-DMA row gather + PE transpose ----
            xgT = [sb.tile([P, cap], bf16, tag=f"xgT{hc}", name=f"xgT{hc}") for hc in range(HC)]
            gtiles = [(k * P, P) for k in range(NFT - 1)] + [((NFT - 1) * P, rem)]
            for gi, (goff, gn) in enumerate(gtiles):
                off_ap = idxp[0:gn, gi:gi + 1]
                xg = sbw.tile([gn, H], bf16, tag="xg", name=f"xg{gi}", bufs=3)
                nc.gpsimd.indirect_dma_start(
                    out=xg[:], out_offset=None,
                    in_=x_d[:],
                    in_offset=bass.IndirectOffsetOnAxis(ap=off_ap, axis=0),
                )
                if gi == 0:
                    wps_b = psg.tile([8, 512], f32, tag="mm3", name="warmxg")
                    nc.tensor.matmul(
                        out=wps_b[:], lhsT=idtb[0:gn, 0:8], rhs=xg[:, 0:512],
                        start=True, stop=True,
                    )
                for hc in range(HC):
                    tpx = psg.tile([P, gn], bf16, tag="mm3", name=f"xtr{gi}{hc}")
                    nc.tensor.transpose(
                        out=tpx[:], in_=xg[:, hc * P:(hc + 1) * P],
                        identity=idtb[0:gn, 0:gn],
                    )
                    nc.vector.tensor_copy(
                        out=xgT[hc][:, goff:goff + gn], in_=tpx[:]
                    )

            # ---- F. coef broadcast [128, cap] via dense DRAM round trip ----
            tpc = psg.tile([NFT, P], f32, tag="mm1", name="tpc")
            nc.tensor.transpose(out=tpc[:], in_=coefp[:], identity=idt[:])
            ct = sb.tile([NFT, P], f32, tag="ct")
            nc.vector.tensor_copy(out=ct[:], in_=tpc[:])
            nc.scalar.dma_start(out=d_cf[:].rearrange("(k p) -> k p", p=P), in_=ct[:])
            vrow = sb.tile([1, cap], f32, tag="vrow")
            nc.scalar.dma_start(out=vrow[:], in_=d_cf[0:cap].rearrange("(o c) -> o c", o=1))
            cbc = sb.tile([P, cap], f32, tag="cbc")
            for g in range(ngrp):
                cb_ps = psg.tile([P, grp], f32, tag="mm2")
                nc.tensor.matmul(
                    out=cb_ps[:], lhsT=onesP[:],
                    rhs=vrow[:, g * grp:(g + 1) * grp], start=True, stop=True,
                )
                nc.vector.tensor_copy(out=cbc[:, g * grp:(g + 1) * grp], in_=cb_ps[:])

            # ---- G. h1 = x@w1, h3 = x@w3 (transposed), fused silu*mul ----
            actT = [sb.tile([P, cap], bf16, tag=f"actT{ic}", name=f"actT{ic}") for ic in range(IC)]
            for ic in range(IC):
                w1_sl = sbw.tile([P, H], bf16, tag="w1sl", bufs=4)
                nc.sync.dma_start(
                    out=w1_sl[:].rearrange("p (hc i) -> p hc i", i=P),
                    in_=w1[:, ic * P:(ic + 1) * P].rearrange("(hc p) i -> p hc i", p=P),
                )
                w3_sl = sbw.tile([P, H], bf16, tag="w3sl", bufs=4)
                nc.sync.dma_start(
                    out=w3_sl[:].rearrange("p (hc i) -> p hc i", i=P),
                    in_=w3[:, ic * P:(ic + 1) * P].rearrange("(hc p) i -> p hc i", p=P),
                )
                for g in range(ngrp):
                    gs = slice(g * grp, (g + 1) * grp)
                    ps1 = psg.tile([P, grp], f32, tag="mm0")
                    ps3 = psg.tile([P, grp], f32, tag="mm1")
                    for hc in range(HC):
                        nc.tensor.matmul(
                            out=ps1[:],
                            lhsT=w1_sl[:, hc * P:(hc + 1) * P],
                            rhs=xgT[hc][:, gs],
                            start=(hc == 0), stop=(hc == HC - 1),
                        )
                    for hc in range(HC):
                        nc.tensor.matmul(
                            out=ps3[:],
                            lhsT=w3_sl[:, hc * P:(hc + 1) * P],
                            rhs=xgT[hc][:, gs],
                            start=(hc == 0), stop=(hc == HC - 1),
                        )
                    sl = sbw.tile([P, grp], f32, tag="silu")
                    nc.scalar.activation(out=sl[:], in_=ps1[:], func=AF.Silu)
                    nc.vector.tensor_mul(out=actT[ic][:, gs], in0=sl[:], in1=ps3[:])

            # ---- H. yT = (act @ w2).T * coef ----
            for hc in range(HC):
                w2_sl = sbw.tile([P, II], bf16, tag="w2sl", bufs=3)
                nc.sync.dma_start(
                    out=w2_sl[:].rearrange("p (ic h) -> p ic h", h=P),
                    in_=w2[:, hc * P:(hc + 1) * P].rearrange("(ic p) h -> p ic h", p=P),
                )
                for g in range(ngrp):
                    gs = slice(g * grp, (g + 1) * grp)
                    pso = psg.tile([P, grp], f32, tag="mm2")
                    for ic in range(IC):
                        nc.tensor.matmul(
                            out=pso[:],
                            lhsT=w2_sl[:, ic * P:(ic + 1) * P],
                            rhs=actT[ic][:, gs],
                            start=(ic == 0), stop=(ic == IC - 1),
                        )
                    yt_sb = sbw.tile([P, grp], f32, tag="yt")
                    nc.vector.tensor_mul(out=yt_sb[:], in0=pso[:], in1=cbc[:, gs])
                    nc.sync.dma_start(
                        out=o_yt[hc * P:(hc + 1) * P, gs], in_=yt_sb[:]
                    )

    nc.compile()
    return nc


def _get_built(cap):
    if cap not in _build_cache:
        _build_cache[cap] = _build(cap)
    return _build_cache[cap]


def _make_consts(cap):
    nft = (cap + P - 1) // P
    tokid_np = (np.arange(NT)[None, :] * P + np.arange(P)[:, None]).astype(np.float32)
    io128_np = np.broadcast_to(
        np.tile(np.arange(P), NT)[None, :], (P, NT * P)
    ).astype(np.float16)
    io5_np = np.broadcast_to(
        np.tile(np.arange(nft), NT)[None, :], (P, NT * nft)
    ).astype(np.float16)
    stril_np = np.triu(np.ones((P, P), np.float32), 1)  # [k, m] = 1 if k < m
    ones128_np = np.ones((P, 1), np.float32)
    ident_np = np.eye(P, dtype=np.float32)
    return tokid_np, io128_np, io5_np, stril_np, ones128_np, ident_np


def _run(cap, hs, gate_w, w1s, w2s, w3s, trace=False):
    import ml_dtypes
    from concourse.bass_utils import run_bass_kernel_spmd

    nc = _get_built(cap)

    bf = ml_dtypes.bfloat16
    x_hi = hs.astype(bf)
    x_lo = (hs - x_hi.astype(np.float32)).astype(bf)
    xth_np = np.ascontiguousarray(x_hi.T)
    xtl_np = np.ascontiguousarray(x_lo.T)
    gw_hi = gate_w.astype(bf)
    gw_lo = (gate_w - gw_hi.astype(np.float32)).astype(bf)
    gws_np = np.zeros((H, 40), np.float32)
    gws_np[:, 0:E] = gw_hi.astype(np.float32)
    gws_np[:, 32:40] = gw_lo.astype(np.float32)
    gws_np = np.ascontiguousarray(gws_np.astype(bf))
    oh_base = np.zeros((P, NT, E), np.float32)
    x_bf = np.ascontiguousarray(x_hi)
    tokid_np, io128_np, io5_np, stril_np, ones128_np, ident_np = _make_consts(cap)

    in_maps = []
    for c in range(NCORES):
        oh_c = oh_base.copy()
        oh_c[:, :, c] = 1.0
        in_maps.append({
            "xth": xth_np,
            "xtl": xtl_np,
            "gws": gws_np,
            "w1": np.ascontiguousarray(w1s[c].astype(bf)),
            "w3": np.ascontiguousarray(w3s[c].astype(bf)),
            "w2": np.ascontiguousarray(w2s[c].astype(bf)),
            "oh": oh_c.reshape(P, NT * E),
            "tokid": tokid_np,
            "tokh": tokid_np.astype(np.float16),
            "x": x_bf,
            "io128": io128_np,
            "io5": io5_np,
            "stril": stril_np.astype(bf),
            "ones128": ones128_np.astype(bf),
            "ident": ident_np,
        })

    res = run_bass_kernel_spmd(nc, in_maps, list(range(NCORES)), trace=trace)
    return res


def kernel(hidden_states, gate_w, w1s, w2s, w3s, _trace=False, _cap=560):
    hs = np.ascontiguousarray(np.asarray(hidden_states, dtype=np.float32))
    gate_w = np.ascontiguousarray(np.asarray(gate_w, dtype=np.float32))
    w1s = np.asarray(w1s, dtype=np.float32)
    w2s = np.asarray(w2s, dtype=np.float32)
    w3s = np.asarray(w3s, dtype=np.float32)

    cap = _cap
    while True:
        res = _run(cap, hs, gate_w, w1s, w2s, w3s, trace=_trace)
        counts = [int(res.results[c]["o_cnt"].ravel()[0]) for c in range(NCORES)]
        if max(counts) <= cap:
            break
        # capacity overflow (won't happen for sane routing): rebuild bigger
        cap = 2048 if max(counts) > 1024 else 1024

    out = np.zeros((T, H), dtype=np.float32)
    for c in range(NCORES):
        r = res.results[c]
        cnt = counts[c]
        idx = r["o_idx"][:cnt]
        y = np.ascontiguousarray(r["o_yt"].T[:cnt])
        out[idx] += y
    kernel._last_results = res
    return out
